# revision 9
# baseline (speedup 1.0000x reference)
"""NetVLAD forward kernel for Trainium2, 8-core data-parallel SPMD.

Problem (hardcoded):
  x         [32, 256, 64, 64] f32
  conv_w    [64, 256] f32
  conv_b    [64] f32
  centroids [64, 256] f32
  out       [32, 64*256] f32

  x_n   = l2norm(x, axis=c)
  a     = softmax(conv_w @ x_n + b, axis=k)         # [n, 64, 4096]
  vlad  = a @ x_n^T - a.sum(s) * centroids          # [n, 64, 256]
  out   = l2norm(l2norm(vlad, axis=c).reshape(n, -1), axis=1)

Sharding: batch n=32 split 4 items per core across 8 cores. Weights
replicated. No collectives; host gathers per-core outputs.

Device algorithm per item (all reductions on the free dim):
  - x shipped in two bf16 layouts: natural [c, s] (GEMM1 stationary) and
    transposed [s, c] (GEMM2 moving + sum-of-squares source).
  - ss_s = sum_c x^2 (DVE tensor_tensor_reduce / ACT square+accum)
  - r = rsqrt(ss) and norm n = sqrt(ss) via exp(+-0.5*ln(ss)) so that the
    scalar engine only ever needs the natural_log_exp_and_others table set.
  - GEMM1 (transposed direct): zT[s,k] = sum_c x[c,s] W[k,c] on PE,
    psum per 128-row s-tile.
  - t = exp(zT * r) on ACT (scale = per-partition r, reading PSUM),
    t2 = t * exp(b) with denominator accumulation (DVE ttr),
    a' = t2 * (r/d) (DVE tensor_scalar)  [a' = softmax * r]
  - GEMM2: [vlad_raw | asum] = a'^T @ [xT | n] accumulated over 32 s-tiles.
    Column 256 gives asum = sum_s softmax (since a' * n = softmax).
  - vlad = vlad_raw - asum*cent; intra-l2norm over c; global norm is
    exactly sqrt(K)=8 after the intra norm, so fold 1/8 into the row scale.
"""

import numpy as np
import ml_dtypes

N_FULL, DIM, HH, WW = 32, 256, 64, 64
K = 64
S = HH * WW            # 4096
NC = 8
NPC = N_FULL // NC     # items per core
ST = S // 128          # s-tiles per item
CW = DIM + 2           # xt tile row width (c + norm column + alignment pad)
NW = DIM + 1           # matmul rhs width actually consumed (c + norm column)
ND = 20                # sumsq tiles handled by DVE; the rest go to ACT

BF16 = ml_dtypes.bfloat16

_CACHE = {}


def _emit(tc, ctx, xb_d, xt_d, wt_d, bb_d, ct_d, out_d, npc):
    import concourse.bass as bass
    from concourse import mybir

    f32 = mybir.dt.float32
    bf16 = mybir.dt.bfloat16
    AF = mybir.ActivationFunctionType
    OP = mybir.AluOpType
    ts = bass.ts
    nc = tc.nc

    if True:
        consts = ctx.enter_context(tc.tile_pool(name="consts", bufs=1))
        xbp = ctx.enter_context(tc.tile_pool(name="xbp", bufs=2))
        xtp = ctx.enter_context(tc.tile_pool(name="xtp", bufs=2))
        t2p = ctx.enter_context(tc.tile_pool(name="t2p", bufs=2))
        sml = ctx.enter_context(tc.tile_pool(name="sml", bufs=2))
        sqp = ctx.enter_context(tc.tile_pool(name="sqp", bufs=3))
        tp = ctx.enter_context(tc.tile_pool(name="tp", bufs=4))
        app = ctx.enter_context(tc.tile_pool(name="app", bufs=4))
        pst = ctx.enter_context(tc.tile_pool(name="pst", bufs=2))
        pzp = ctx.enter_context(tc.tile_pool(name="pzp", bufs=4, space="PSUM"))
        pvp = ctx.enter_context(tc.tile_pool(name="pvp", bufs=2, space="PSUM"))

        # ---- one-time constants ----
        wt_sb = consts.tile([128, 2, K], bf16)
        nc.sync.dma_start(out=wt_sb[:], in_=wt_d[:, :, :])
        ct_sb = consts.tile([K, DIM], f32)
        nc.sync.dma_start(out=ct_sb[:], in_=ct_d[:, :])
        # conv_b broadcast across partitions, then eb = exp(b) in bf16
        bbc = consts.tile([128, K], f32)
        bb_bcast = bass.AP(tensor=bb_d.tensor, offset=bb_d.offset, ap=[[0, 128], [1, K]])
        nc.gpsimd.dma_start(out=bbc[:], in_=bb_bcast)
        eb_f = consts.tile([128, K], f32)
        nc.scalar.activation(eb_f[:], bbc[:], AF.Exp)
        eb = consts.tile([128, K], bf16)
        nc.vector.tensor_copy(eb[:], eb_f[:])

        for i in range(npc):
            # ---- loads ----
            xb = xbp.tile([128, 2, S], bf16)
            nc.sync.dma_start(out=xb[:], in_=xb_d[i, :, :, :])
            xt = xtp.tile([128, ST, CW], bf16)
            nc.sync.dma_start(out=xt[:, :, 0:DIM], in_=xt_d[i, :, :, :])

            # ---- sum of squares over c (split DVE / ACT) ----
            ss_d = sml.tile([128, ND], f32)
            ss_a = sml.tile([128, ST - ND], f32)
            for j in range(ST):
                sq = sqp.tile([128, DIM], bf16, tag="sq")
                if j < ND:
                    nc.vector.scalar_tensor_tensor(
                        out=sq[:],
                        in0=xt[:, j, 0:DIM],
                        scalar=1.0,
                        in1=xt[:, j, 0:DIM],
                        op0=OP.mult,
                        op1=OP.mult,
                        accum_out=ss_d[:, j : j + 1],
                    )
                else:
                    nc.scalar.activation(
                        sq[:],
                        xt[:, j, 0:DIM],
                        AF.Square,
                        accum_out=ss_a[:, j - ND : j - ND + 1],
                    )

            # r = ss^-0.5, n = ss^0.5 via ln/exp (stays in one ACT table set)
            ln_ss = sml.tile([128, ST], f32)
            nc.scalar.activation(ln_ss[:, 0:ND], ss_d[:], AF.Ln)
            nc.scalar.activation(ln_ss[:, ND:ST], ss_a[:], AF.Ln)
            r_all = sml.tile([128, ST], f32)
            nc.scalar.activation(r_all[:], ln_ss[:], AF.Exp, scale=-0.5)
            n_all = sml.tile([128, ST], f32)
            nc.scalar.activation(n_all[:], ln_ss[:], AF.Exp, scale=0.5)
            n_bf = sml.tile([128, ST], bf16)
            nc.vector.tensor_copy(n_bf[:], n_all[:])
            # write norms into column 256 of each xt s-tile (GEMM2 asum col)
            nc.vector.tensor_copy(
                xt[:, :, DIM : DIM + 1], n_bf[:].rearrange("p (t o) -> p t o", o=1)
            )

            # ---- GEMM1 (transposed) + softmax numerator/denominator ----
            d_all = sml.tile([128, ST], f32)
            t2 = t2p.tile([128, ST, K], bf16)
            for j in range(ST):
                pz = pzp.tile([128, K], f32, tag="pz")
                nc.tensor.matmul(
                    pz[:], xb[:, 0, ts(j, 128)], wt_sb[:, 0, :], start=True, stop=False
                )
                nc.tensor.matmul(
                    pz[:], xb[:, 1, ts(j, 128)], wt_sb[:, 1, :], start=False, stop=True
                )
                t = tp.tile([128, K], bf16, tag="t")
                nc.scalar.activation(
                    t[:], pz[:], AF.Exp, scale=r_all[:, j : j + 1]
                )
                nc.vector.scalar_tensor_tensor(
                    out=t2[:, j, :],
                    in0=t[:],
                    scalar=1.0,
                    in1=eb[:],
                    op0=OP.mult,
                    op1=OP.mult,
                    accum_out=d_all[:, j : j + 1],
                )

            rd = sml.tile([128, ST], f32)
            nc.vector.reciprocal(rd[:], d_all[:])
            rdr = sml.tile([128, ST], f32)
            nc.vector.tensor_mul(rdr[:], rd[:], r_all[:])

            # ---- a' = t2 * (r/d), GEMM2 accumulation ----
            pv = pvp.tile([K, NW], f32, tag="pv")
            for j in range(ST):
                ap = app.tile([128, K], bf16, tag="ap")
                nc.vector.tensor_scalar_mul(ap[:], t2[:, j, :], rdr[:, j : j + 1])
                nc.tensor.matmul(
                    pv[:], ap[:], xt[:, j, 0:NW], start=(j == 0), stop=(j == ST - 1)
                )

            # ---- epilogue: centroid correction + intra norm + 1/8 ----
            nasum = sml.tile([K, 1], f32)
            nc.vector.tensor_scalar_mul(nasum[:], pv[:, DIM : DIM + 1], -1.0)
            v2 = pst.tile([K, DIM], f32, tag="v2")
            nc.vector.scalar_tensor_tensor(
                out=v2[:],
                in0=ct_sb[:],
                scalar=nasum[:],
                in1=pv[:, 0:DIM],
                op0=OP.mult,
                op1=OP.add,
            )
            scrv = pst.tile([K, DIM], f32, tag="scrv")
            ssv = sml.tile([K, 1], f32)
            nc.vector.scalar_tensor_tensor(
                out=scrv[:],
                in0=v2[:],
                scalar=1.0,
                in1=v2[:],
                op0=OP.mult,
                op1=OP.mult,
                accum_out=ssv[:],
            )
            inv = sml.tile([K, 1], f32)
            nc.vector.reciprocal(inv[:], ssv[:])
            lnv = sml.tile([K, 1], f32)
            nc.scalar.activation(lnv[:], inv[:], AF.Ln)
            scl = sml.tile([K, 1], f32)
            # exp(0.5*ln(1/ss)) = rsqrt(ss); the global l2 norm after the
            # intra norm is exactly sqrt(K)=8, folded in as *0.125 below.
            nc.scalar.activation(scl[:], lnv[:], AF.Exp, scale=0.5)
            osb = pst.tile([K, DIM], f32, tag="osb")
            nc.vector.tensor_scalar(
                out=osb[:], in0=v2[:], scalar1=scl[:], scalar2=0.125,
                op0=OP.mult, op1=OP.mult,
            )
            nc.sync.dma_start(out=out_d[i, :, :], in_=osb[:])


def _build_program():
    from contextlib import ExitStack
    import concourse.tile as tile
    from concourse import bacc, mybir

    f32 = mybir.dt.float32
    bf16 = mybir.dt.bfloat16

    nc = bacc.Bacc(
        "TRN2", target_bir_lowering=False, debug=False, enable_asserts=False
    )

    xb_d = nc.dram_tensor("xb", [NPC, 128, 2, S], bf16, kind="ExternalInput").ap()
    xt_d = nc.dram_tensor("xt", [NPC, 128, ST, DIM], bf16, kind="ExternalInput").ap()
    wt_d = nc.dram_tensor("wt", [128, 2, K], bf16, kind="ExternalInput").ap()
    bb_d = nc.dram_tensor("bb", [1, K], f32, kind="ExternalInput").ap()
    ct_d = nc.dram_tensor("ct", [K, DIM], f32, kind="ExternalInput").ap()
    out_d = nc.dram_tensor("out", [NPC, K, DIM], f32, kind="ExternalOutput").ap()

    with tile.TileContext(nc) as tc, ExitStack() as ctx:
        _emit(tc, ctx, xb_d, xt_d, wt_d, bb_d, ct_d, out_d, NPC)

    nc.compile()
    return nc


def _get_program():
    if "nc" not in _CACHE:
        _CACHE["nc"] = _build_program()
    return _CACHE["nc"]


def _prep_inputs(x, conv_w, conv_b, centroids):
    xf = np.asarray(x, dtype=np.float32).reshape(N_FULL, DIM, S)
    # natural layout [n, p, u, s]: xb[i, p, u, s] = x[i, 128u+p, s]
    xb = np.ascontiguousarray(
        xf.reshape(N_FULL, 2, 128, S).transpose(0, 2, 1, 3)
    ).astype(BF16)
    # transposed layout [n, p, t, c]: xt[i, p, t, c] = x[i, c, 128t+p]
    xt = np.ascontiguousarray(
        xf.transpose(0, 2, 1).reshape(N_FULL, ST, 128, DIM).transpose(0, 2, 1, 3)
    ).astype(BF16)
    # wt[p, u, k] = conv_w[k, 128u+p]
    wt = np.ascontiguousarray(
        np.asarray(conv_w, dtype=np.float32).T.reshape(2, 128, K).transpose(1, 0, 2)
    ).astype(BF16)
    bb = np.asarray(conv_b, dtype=np.float32).reshape(1, K)
    ct = np.ascontiguousarray(np.asarray(centroids, dtype=np.float32))
    in_maps = []
    for c in range(NC):
        sl = slice(c * NPC, (c + 1) * NPC)
        in_maps.append(
            {
                "xb": np.ascontiguousarray(xb[sl]),
                "xt": np.ascontiguousarray(xt[sl]),
                "wt": wt,
                "bb": bb,
                "ct": ct,
            }
        )
    return in_maps


def kernel(x, conv_w, conv_b, centroids):
    from concourse.bass_utils import run_bass_kernel_spmd

    nc = _get_program()
    in_maps = _prep_inputs(x, conv_w, conv_b, centroids)
    res = run_bass_kernel_spmd(nc, in_maps, core_ids=list(range(NC)))
    outs = [res.results[c]["out"].reshape(NPC, K * DIM) for c in range(NC)]
    return np.concatenate(outs, axis=0)


# revision 12
# speedup vs baseline: 88.0727x; 88.0727x over previous
"""NetVLAD forward kernel for Trainium2, 8-core data-parallel SPMD.

Problem (hardcoded):
  x         [32, 256, 64, 64] f32
  conv_w    [64, 256] f32
  conv_b    [64] f32
  centroids [64, 256] f32
  out       [32, 64*256] f32

  x_n   = l2norm(x, axis=c)
  a     = softmax(conv_w @ x_n + b, axis=k)         # [n, 64, 4096]
  vlad  = a @ x_n^T - a.sum(s) * centroids          # [n, 64, 256]
  out   = l2norm(l2norm(vlad, axis=c).reshape(n, -1), axis=1)

Sharding: batch n=32 split 4 items per core across 8 cores. Weights
replicated. No collectives; host gathers per-core outputs.

Device algorithm per item (all reductions on the free dim):
  - x shipped in two bf16 layouts: natural [c, s] (GEMM1 stationary) and
    transposed [s, c] (GEMM2 moving + sum-of-squares source).
  - ss_s = sum_c x^2 (DVE tensor_tensor_reduce / ACT square+accum)
  - r = rsqrt(ss) and norm n = sqrt(ss) via exp(+-0.5*ln(ss)) so that the
    scalar engine only ever needs the natural_log_exp_and_others table set.
  - GEMM1 (transposed direct): zT[s,k] = sum_c x[c,s] W[k,c] on PE,
    psum per 128-row s-tile.
  - t = exp(zT * r) on ACT (scale = per-partition r, reading PSUM),
    t2 = t * exp(b) with denominator accumulation (DVE ttr),
    a' = t2 * (r/d) (DVE tensor_scalar)  [a' = softmax * r]
  - GEMM2: [vlad_raw | asum] = a'^T @ [xT | n] accumulated over 32 s-tiles.
    Column 256 gives asum = sum_s softmax (since a' * n = softmax).
  - vlad = vlad_raw - asum*cent; intra-l2norm over c; global norm is
    exactly sqrt(K)=8 after the intra norm, so fold 1/8 into the row scale.
"""

import numpy as np
import ml_dtypes

N_FULL, DIM, HH, WW = 32, 256, 64, 64
K = 64
S = HH * WW            # 4096
NC = 8
NPC = N_FULL // NC     # items per core
ST = S // 128          # s-tiles per item
CW = DIM + 2           # xt tile row width (c + norm column + alignment pad)
NW = DIM + 1           # matmul rhs width actually consumed (c + norm column)
ND = 20                # sumsq tiles handled by DVE; the rest go to ACT

BF16 = ml_dtypes.bfloat16

_CACHE = {}


def _emit(tc, ctx, xb_d, xt_d, wt_d, bb_d, ct_d, out_d, npc, repeat=1):
    import concourse.bass as bass
    from concourse import mybir

    f32 = mybir.dt.float32
    bf16 = mybir.dt.bfloat16
    AF = mybir.ActivationFunctionType
    OP = mybir.AluOpType
    ts = bass.ts
    nc = tc.nc

    if True:
        consts = ctx.enter_context(tc.tile_pool(name="consts", bufs=1))
        xbp = ctx.enter_context(tc.tile_pool(name="xbp", bufs=2))
        xtp = ctx.enter_context(tc.tile_pool(name="xtp", bufs=2))
        t2p = ctx.enter_context(tc.tile_pool(name="t2p", bufs=2))
        sml = ctx.enter_context(tc.tile_pool(name="sml", bufs=2))
        sqp = ctx.enter_context(tc.tile_pool(name="sqp", bufs=3))
        tp = ctx.enter_context(tc.tile_pool(name="tp", bufs=4))
        app = ctx.enter_context(tc.tile_pool(name="app", bufs=4))
        pst = ctx.enter_context(tc.tile_pool(name="pst", bufs=2))
        pzp = ctx.enter_context(tc.tile_pool(name="pzp", bufs=4, space="PSUM"))
        pvp = ctx.enter_context(tc.tile_pool(name="pvp", bufs=2, space="PSUM"))

        # ---- one-time constants ----
        wt_sb = consts.tile([128, 2, K], bf16)
        nc.sync.dma_start(out=wt_sb[:], in_=wt_d[:, :, :])
        ct_sb = consts.tile([K, DIM], f32)
        nc.sync.dma_start(out=ct_sb[:], in_=ct_d[:, :])
        # conv_b broadcast across partitions, then eb = exp(b) in bf16
        bbc = consts.tile([128, K], f32)
        bb_bcast = bass.AP(tensor=bb_d.tensor, offset=bb_d.offset, ap=[[0, 128], [1, K]])
        nc.gpsimd.dma_start(out=bbc[:], in_=bb_bcast)
        eb_f = consts.tile([128, K], f32)
        nc.scalar.activation(eb_f[:], bbc[:], AF.Exp)
        eb = consts.tile([128, K], bf16)
        nc.vector.tensor_copy(eb[:], eb_f[:])

        if repeat > 1:
            # timing builds: loop the whole per-item body inside the NEFF so
            # device time dominates the per-dispatch tunnel overhead
            ctx.enter_context(tc.For_i(0, repeat, 1))

        for i in range(npc):
            # ---- loads ----
            xb = xbp.tile([128, 2, S], bf16)
            nc.sync.dma_start(out=xb[:], in_=xb_d[i, :, :, :])
            xt = xtp.tile([128, ST, CW], bf16)
            nc.sync.dma_start(out=xt[:, :, 0:DIM], in_=xt_d[i, :, :, :])

            # ---- sum of squares over c (split DVE / ACT) ----
            ss_d = sml.tile([128, ND], f32)
            ss_a = sml.tile([128, ST - ND], f32)
            for j in range(ST):
                sq = sqp.tile([128, DIM], bf16, tag="sq")
                if j < ND:
                    nc.vector.scalar_tensor_tensor(
                        out=sq[:],
                        in0=xt[:, j, 0:DIM],
                        scalar=1.0,
                        in1=xt[:, j, 0:DIM],
                        op0=OP.mult,
                        op1=OP.mult,
                        accum_out=ss_d[:, j : j + 1],
                    )
                else:
                    nc.scalar.activation(
                        sq[:],
                        xt[:, j, 0:DIM],
                        AF.Square,
                        accum_out=ss_a[:, j - ND : j - ND + 1],
                    )

            # r = ss^-0.5, n = ss^0.5 via ln/exp (stays in one ACT table set)
            ln_ss = sml.tile([128, ST], f32)
            nc.scalar.activation(ln_ss[:, 0:ND], ss_d[:], AF.Ln)
            nc.scalar.activation(ln_ss[:, ND:ST], ss_a[:], AF.Ln)
            r_all = sml.tile([128, ST], f32)
            nc.scalar.activation(r_all[:], ln_ss[:], AF.Exp, scale=-0.5)
            n_all = sml.tile([128, ST], f32)
            nc.scalar.activation(n_all[:], ln_ss[:], AF.Exp, scale=0.5)
            n_bf = sml.tile([128, ST], bf16)
            nc.vector.tensor_copy(n_bf[:], n_all[:])
            # write norms into column 256 of each xt s-tile (GEMM2 asum col)
            nc.vector.tensor_copy(
                xt[:, :, DIM : DIM + 1], n_bf[:].rearrange("p (t o) -> p t o", o=1)
            )

            # ---- GEMM1 (transposed) + softmax numerator/denominator ----
            d_all = sml.tile([128, ST], f32)
            t2 = t2p.tile([128, ST, K], bf16)
            for j in range(ST):
                pz = pzp.tile([128, K], f32, tag="pz")
                nc.tensor.matmul(
                    pz[:], xb[:, 0, ts(j, 128)], wt_sb[:, 0, :], start=True, stop=False
                )
                nc.tensor.matmul(
                    pz[:], xb[:, 1, ts(j, 128)], wt_sb[:, 1, :], start=False, stop=True
                )
                t = tp.tile([128, K], bf16, tag="t")
                nc.scalar.activation(
                    t[:], pz[:], AF.Exp, scale=r_all[:, j : j + 1]
                )
                nc.vector.scalar_tensor_tensor(
                    out=t2[:, j, :],
                    in0=t[:],
                    scalar=1.0,
                    in1=eb[:],
                    op0=OP.mult,
                    op1=OP.mult,
                    accum_out=d_all[:, j : j + 1],
                )

            rd = sml.tile([128, ST], f32)
            nc.vector.reciprocal(rd[:], d_all[:])
            rdr = sml.tile([128, ST], f32)
            nc.vector.tensor_mul(rdr[:], rd[:], r_all[:])

            # ---- a' = t2 * (r/d), GEMM2 accumulation ----
            pv = pvp.tile([K, NW], f32, tag="pv")
            for j in range(ST):
                ap = app.tile([128, K], bf16, tag="ap")
                nc.vector.tensor_scalar_mul(ap[:], t2[:, j, :], rdr[:, j : j + 1])
                nc.tensor.matmul(
                    pv[:], ap[:], xt[:, j, 0:NW], start=(j == 0), stop=(j == ST - 1)
                )

            # ---- epilogue: centroid correction + intra norm + 1/8 ----
            nasum = sml.tile([K, 1], f32)
            nc.vector.tensor_scalar_mul(nasum[:], pv[:, DIM : DIM + 1], -1.0)
            v2 = pst.tile([K, DIM], f32, tag="v2")
            nc.vector.scalar_tensor_tensor(
                out=v2[:],
                in0=ct_sb[:],
                scalar=nasum[:],
                in1=pv[:, 0:DIM],
                op0=OP.mult,
                op1=OP.add,
            )
            scrv = pst.tile([K, DIM], f32, tag="scrv")
            ssv = sml.tile([K, 1], f32)
            nc.vector.scalar_tensor_tensor(
                out=scrv[:],
                in0=v2[:],
                scalar=1.0,
                in1=v2[:],
                op0=OP.mult,
                op1=OP.mult,
                accum_out=ssv[:],
            )
            inv = sml.tile([K, 1], f32)
            nc.vector.reciprocal(inv[:], ssv[:])
            lnv = sml.tile([K, 1], f32)
            nc.scalar.activation(lnv[:], inv[:], AF.Ln)
            scl = sml.tile([K, 1], f32)
            # exp(0.5*ln(1/ss)) = rsqrt(ss); the global l2 norm after the
            # intra norm is exactly sqrt(K)=8, folded in as *0.125 below.
            nc.scalar.activation(scl[:], lnv[:], AF.Exp, scale=0.5)
            osb = pst.tile([K, DIM], f32, tag="osb")
            nc.vector.tensor_scalar(
                out=osb[:], in0=v2[:], scalar1=scl[:], scalar2=0.125,
                op0=OP.mult, op1=OP.mult,
            )
            nc.sync.dma_start(out=out_d[i, :, :], in_=osb[:])


def _build_program(repeat=1):
    from contextlib import ExitStack
    import concourse.tile as tile
    from concourse import bacc, mybir

    f32 = mybir.dt.float32
    bf16 = mybir.dt.bfloat16

    nc = bacc.Bacc(
        "TRN2", target_bir_lowering=False, debug=False, enable_asserts=False
    )

    xb_d = nc.dram_tensor("xb", [NPC, 128, 2, S], bf16, kind="ExternalInput").ap()
    xt_d = nc.dram_tensor("xt", [NPC, 128, ST, DIM], bf16, kind="ExternalInput").ap()
    wt_d = nc.dram_tensor("wt", [128, 2, K], bf16, kind="ExternalInput").ap()
    bb_d = nc.dram_tensor("bb", [1, K], f32, kind="ExternalInput").ap()
    ct_d = nc.dram_tensor("ct", [K, DIM], f32, kind="ExternalInput").ap()
    out_d = nc.dram_tensor("out", [NPC, K, DIM], f32, kind="ExternalOutput").ap()

    with tile.TileContext(nc) as tc, ExitStack() as ctx:
        _emit(tc, ctx, xb_d, xt_d, wt_d, bb_d, ct_d, out_d, NPC, repeat=repeat)

    nc.compile()
    return nc


def _get_program():
    if "nc" not in _CACHE:
        _CACHE["nc"] = _build_program()
    return _CACHE["nc"]


def _prep_inputs(x, conv_w, conv_b, centroids):
    xf = np.asarray(x, dtype=np.float32).reshape(N_FULL, DIM, S)
    # natural layout [n, p, u, s]: xb[i, p, u, s] = x[i, 128u+p, s]
    xb = np.ascontiguousarray(
        xf.reshape(N_FULL, 2, 128, S).transpose(0, 2, 1, 3)
    ).astype(BF16)
    # transposed layout [n, p, t, c]: xt[i, p, t, c] = x[i, c, 128t+p]
    xt = np.ascontiguousarray(
        xf.transpose(0, 2, 1).reshape(N_FULL, ST, 128, DIM).transpose(0, 2, 1, 3)
    ).astype(BF16)
    # wt[p, u, k] = conv_w[k, 128u+p]
    wt = np.ascontiguousarray(
        np.asarray(conv_w, dtype=np.float32).T.reshape(2, 128, K).transpose(1, 0, 2)
    ).astype(BF16)
    bb = np.asarray(conv_b, dtype=np.float32).reshape(1, K)
    ct = np.ascontiguousarray(np.asarray(centroids, dtype=np.float32))
    in_maps = []
    for c in range(NC):
        sl = slice(c * NPC, (c + 1) * NPC)
        in_maps.append(
            {
                "xb": np.ascontiguousarray(xb[sl]),
                "xt": np.ascontiguousarray(xt[sl]),
                "wt": wt,
                "bb": bb,
                "ct": ct,
            }
        )
    return in_maps


def kernel(x, conv_w, conv_b, centroids):
    from concourse.bass_utils import run_bass_kernel_spmd

    nc = _get_program()
    in_maps = _prep_inputs(x, conv_w, conv_b, centroids)
    res = run_bass_kernel_spmd(nc, in_maps, core_ids=list(range(NC)))
    outs = [res.results[c]["out"].reshape(NPC, K * DIM) for c in range(NC)]
    return np.concatenate(outs, axis=0)
